# revision 21
# baseline (speedup 1.0000x reference)
"""Trainium2 Bass kernel for nn_AttentionGenerator (gnn_message_passing).

Reference math:
    f = einsum('oc,bctv->botv', Wf, feat) + bf          # 1x1 conv, Cout=64
    s_i = einsum('c,bctv->btv', Wa[:64], f)
    s_j = einsum('c,bctv->btv', Wa[64:], f)
    score[b,t,i,j] = s_i[b,t,i] + s_j[b,t,j] + ba
    atten = (exp(leaky_relu(score)) * A) / row_sum

f only enters through the two dot products, so fold Wf/bf/Wa/ba on the
host into u1 = w1@Wf, u2 = w2@Wf (length-256 vectors) and the scalar
c0 = (w1+w2)@bf + ba.  The device computes, per (b,t,v), the two
channel contractions (TensorEngine), an 18x18 broadcast-add + LeakyReLU
+ exp*A + row-normalize.  Memory bound.

Key layout trick ("grouped rotation matmul"): the 128 contraction
partitions are split into G=8 groups of 16.  Each group g owns a
different 1/8 of the (t,v) columns, and over NP=8 accumulation passes
the c-subchunks rotate through the groups (rotation pre-baked into the
host data layout so device APs stay affine; stationary weights are
block-diagonal).  The matmul result s then lands in PSUM as [16, 576]
per batch instead of [2, 4608], which makes the PSUM->SBUF evacuation
8x cheaper (engine cost is free-dim cycles).

Numerics: feat and the folded weights stream in fp8e4 (TRN E4M3), which
halves the dominant HBM stream AND doubles PE throughput via the
DoubleRow perf mode (2 contraction rows/cycle).  Weights are pre-scaled
by 2^11 to clear the fp8 subnormal range; the inverse scale and the
folded bias c0/2 ride the PSUM->SBUF activation copy for free.
Accumulation is fp32 in PSUM; intermediates are bf16; output is written
bf16 and upcast to f32 on the host.

Sharding: pure data parallel - batch B=32 split across 8 NeuronCores
(4 batches each), tiny params replicated, no cross-core comms.
"""

import json
import numpy as np
from contextlib import ExitStack

B, Cin, T, V = 32, 256, 256, 18
NCORES = 8
BPC = B // NCORES  # batches per core
G = 8       # partition groups (16 partitions each)
NP = 8      # rotation passes (c-subchunks of 32 = 16 partitions x 2 fp8 pair)
M = 288     # moving columns per (pass, t-half) = 16 t16 * 18 v
WSCALE = 2048.0  # 2^11 weight pre-scale to clear fp8e4 subnormals

_cached_nc = None


def _legalize_waits_json(bir_json):
    """Split instructions carrying >1 sync wait into single-wait NoOps plus
    the original instruction.  The walrus build in this container accepts at
    most ONE sync-wait command per instruction struct; concourse's Tile
    scheduler freely attaches several.  Hoisting the extra waits onto NoOps
    immediately before the instruction (same engine stream, same position)
    preserves semantics exactly - engines execute their stream in order."""
    bir = json.loads(bir_json)
    ctr = 0
    for fn in bir.get("functions", []):
        for blk in fn.get("blocks", []):
            insts = blk.get("instructions")
            if not insts:
                continue
            out = []
            for inst in insts:
                si = inst.get("sync_info") or {}
                waits = si.get("on_wait") or []
                if len(waits) > 1:
                    for w in waits[:-1]:
                        out.append(
                            {
                                "engine": inst.get("engine"),
                                "ins": [],
                                "name": f"wsplit-{ctr}",
                                "opcode": "NoOp",
                                "outs": [],
                                "sync_info": {"on_update": [], "on_wait": [w]},
                            }
                        )
                        ctr += 1
                    si = dict(si)
                    si["on_wait"] = [waits[-1]]
                    inst = dict(inst)
                    inst["sync_info"] = si
                out.append(inst)
            blk["instructions"] = out
    return json.dumps(bir).encode()


_wait_patch_done = False


def _install_wait_legalizer():
    global _wait_patch_done
    if _wait_patch_done:
        return
    import concourse.bass_utils as bass_utils
    import concourse.bass2jax as bass2jax

    orig = bass_utils.compile_bir_kernel

    def wrapped(bir_json, tmpdir, neff_name="file.neff"):
        return orig(_legalize_waits_json(bir_json), tmpdir, neff_name)

    bass_utils.compile_bir_kernel = wrapped
    bass2jax.compile_bir_kernel = wrapped
    _wait_patch_done = True


def _build_nc(c0_half):
    import concourse.bass as bass
    import concourse.mybir as mybir
    import concourse.tile as tile
    from concourse.alu_op_type import AluOpType

    f32 = mybir.dt.float32
    bf16 = mybir.dt.bfloat16
    fp8 = mybir.dt.float8e4
    nc = bass.Bass(num_swdge_queues=4)
    # feat packed on host: [b, part=(g,c16), pass, pair, thalf, m=(t16,v)]
    feat = nc.dram_tensor("feat", [BPC, 128, NP, 2, 2, M], fp8, kind="ExternalInput")
    # block-diagonal rotated weights: [part=(g,c16), pass, pair, col=(g',o)]
    wmat = nc.dram_tensor("wmat", [128, NP, 2, 2 * G], fp8, kind="ExternalInput")
    amat = nc.dram_tensor("amat", [1, V * V], bf16, kind="ExternalInput")
    out = nc.dram_tensor("out", [BPC, T, V, V], bf16, kind="ExternalOutput")

    with ExitStack() as ctx:
        tc = ctx.enter_context(tile.TileContext(nc))
        singles = ctx.enter_context(tc.tile_pool(name="singles", bufs=1))
        fpool = ctx.enter_context(tc.tile_pool(name="fpool", bufs=1))
        pspool = ctx.enter_context(tc.tile_pool(name="pspool", bufs=2, space="PSUM"))
        spool = ctx.enter_context(tc.tile_pool(name="spool", bufs=2))
        tpool = ctx.enter_context(tc.tile_pool(name="tpool", bufs=2))
        work = ctx.enter_context(tc.tile_pool(name="work", bufs=3))
        opool = ctx.enter_context(tc.tile_pool(name="opool", bufs=2))

        w_t = singles.tile([128, NP, 2, 2 * G], fp8)
        nc.sync.dma_start(out=w_t, in_=wmat[:, :, :, :])
        a_bc = singles.tile([128, V * V], bf16)
        nc.sync.dma_start(out=a_bc, in_=amat[0, :].partition_broadcast(128))

        def stage_feat(b):
            # per-batch feat arrives as independent pass-range tiles so each
            # matmul pass can start as soon as its slice lands; batch 0 uses
            # quarter tiles (the PE then starts during the DMA ramp-up)
            ranges = [(0, 2), (2, 2), (4, 4)] if b == 0 else [(0, 4), (4, 4)]
            tiles = []
            for qi, (p0, np_) in enumerate(ranges):
                f_q = fpool.tile(
                    [128, np_, 2, 2, M], fp8, tag=f"f_{b}_{qi}", name=f"f_{b}_{qi}"
                )
                nc.sync.dma_start(out=f_q, in_=feat[b, :, p0 : p0 + np_])
                tiles.append((p0, np_, f_q))
            return tiles

        def _f_slice(st, p):
            for p0, np_, f_q in st["f_t"]:
                if p0 <= p < p0 + np_:
                    return f_q, p - p0
            raise AssertionError(p)

        def stage_matmul(st):
            ps = pspool.tile([2 * G, 2, 512], f32, tag="ps")
            for p in range(NP):
                f_q, pl = _f_slice(st, p)
                for tb in range(2):
                    nc.tensor.matmul(
                        out=ps[:, tb, 0:M],
                        lhsT=w_t[:, p],
                        rhs=f_q[:, pl, :, tb],
                        start=(p == 0),
                        stop=(p == NP - 1),
                        perf_mode=mybir.MatmulPerfMode.DoubleRow,
                    )
            st["ps"] = ps

        def stage_evac(st):
            """PSUM -> SBUF, folding the 2^-11 weight-scale undo and c0/2
            (each of s1,s2 carries half so their sum carries c0).  The copy's
            APs also reorder free dims (tb,tt,v) -> (tt,tb,v) so the scatter
            can group (g tt) into partitions."""
            s12 = spool.tile([2 * G, 16, 2, V], bf16, tag="s12")
            ps = st["ps"]
            psr = bass.AP(
                tensor=ps.tensor,
                offset=ps.offset,
                ap=[ps.ap[0], [V, 16], [512, 2], [1, V]],
            )
            nc.scalar.activation(
                out=s12,
                in_=psr,
                func=mybir.ActivationFunctionType.Copy,
                scale=1.0 / WSCALE,
                bias=c0_half,
            )
            st["s12"] = s12

        def stage_scatter(st):
            """SBUF scatter [16=(o,g), (t16,tb,v)] -> [128=(g,t16), (o,tb,v)]
            on the ACT HWDGE ring (the SWDGE/Q7 path generates these 128
            36B-row descriptors far too slowly - ~4us per batch)."""
            s12t = tpool.tile([128, 2, 2, V], bf16, tag="s12t")
            for o in range(2):
                # in [8(g), 16(tt), 2(tb), 18(v)] -> out [128(g,tt), 2, 18]:
                # flat element orders match, the DMA pairs them up.
                nc.scalar.dma_start(
                    out=s12t[:, o], in_=st["s12"][o * G : (o + 1) * G]
                )
            st["s12t"] = s12t

        def stage_score(st):
            """score = (s1'+c0/2) + (s2'+c0/2) broadcast-add, then LeakyReLU,
            on DVE."""
            s12t = st["s12t"]
            sc = work.tile([128, 2, V, V], bf16, tag="sc")
            s1b = bass.AP(
                tensor=s12t.tensor,
                offset=s12t.offset,
                ap=[s12t.ap[0], [V, 2], [1, V], [0, V]],
            )
            s2b = bass.AP(
                tensor=s12t.tensor,
                offset=s12t.offset + 2 * V,
                ap=[s12t.ap[0], [V, 2], [0, V], [1, V]],
            )
            nc.vector.tensor_add(out=sc, in0=s1b, in1=s2b)
            lk = work.tile([128, 2 * V * V], bf16, tag="lk")
            scf = bass.AP(
                tensor=sc.tensor, offset=sc.offset, ap=[sc.ap[0], [1, 2 * V * V]]
            )
            nc.vector.scalar_tensor_tensor(
                out=lk,
                in0=scf,
                scalar=0.1,
                in1=scf,
                op0=AluOpType.mult,
                op1=AluOpType.max,
            )
            st["lk"] = lk

        def stage_exp(st):
            ex = work.tile([128, 2 * V * V], bf16, tag="ex")
            nc.scalar.activation(
                out=ex, in_=st["lk"], func=mybir.ActivationFunctionType.Exp
            )
            st["ex"] = ex

        def stage_mask(st, on_pool):
            """exa = ex * A (adjacency mask)."""
            ex = st["ex"]
            exa = work.tile([128, 2, V * V], bf16, tag="exa")
            exv = bass.AP(
                tensor=ex.tensor, offset=ex.offset, ap=[ex.ap[0], [V * V, 2], [1, V * V]]
            )
            abc = bass.AP(
                tensor=a_bc.tensor,
                offset=a_bc.offset,
                ap=[a_bc.ap[0], [0, 2], [1, V * V]],
            )
            eng = nc.gpsimd if on_pool else nc.vector
            eng.tensor_mul(out=exa, in0=exv, in1=abc)
            st["exa"] = exa

        def stage_sum(st):
            """segmented row-sum + reciprocal on DVE."""
            exa = st["exa"]
            exr = bass.AP(
                tensor=exa.tensor,
                offset=exa.offset,
                ap=[exa.ap[0], [V, 2 * V], [1, V]],
            )
            ssum = work.tile([128, 2 * V], f32, tag="ssum")
            nc.vector.reduce_sum(out=ssum, in_=exr, axis=mybir.AxisListType.X)
            rec = work.tile([128, 2 * V], bf16, tag="rec")
            with nc.allow_low_precision(reason="bf16 reciprocal is plenty here"):
                nc.vector.reciprocal(out=rec, in_=ssum)
            st["rec"] = rec

        def stage_norm(st, on_pool):
            """normalize: att = exa * rec-broadcast (Pool in steady state to
            offload DVE; DVE for the tail batch to shorten the critical
            path)."""
            exa, rec = st["exa"], st["rec"]
            att = opool.tile([128, 2, V, V], bf16, tag="att")
            exr2 = bass.AP(
                tensor=exa.tensor,
                offset=exa.offset,
                ap=[exa.ap[0], [V * V, 2], [V, V], [1, V]],
            )
            rb = bass.AP(
                tensor=rec.tensor,
                offset=rec.offset,
                ap=[rec.ap[0], [V, 2], [1, V], [0, V]],
            )
            eng = nc.gpsimd if on_pool else nc.vector
            eng.tensor_mul(out=att, in0=exr2, in1=rb)
            st["att"] = att

        def stage_out(st):
            # sync HWDGE ring: idle once the feat prefetch has drained
            nc.sync.dma_start(
                out=out[st["b"], :, :, :].rearrange("(tb p) i j -> p tb i j", p=128),
                in_=st["att"],
            )

        # --- tb-granular stages for the last batch (shorter tail) ---

        def stage_matmul_tb(st, tb):
            # separate PSUM tile per tb so the two t-half pipelines carry no
            # artificial tile-granular WAR dependencies
            ps = pspool.tile([2 * G, 512], f32, tag=f"ps_tb{tb}", name=f"ps_tb{tb}")
            st[f"ps{tb}"] = ps
            for p in range(NP):
                f_q, pl = _f_slice(st, p)
                nc.tensor.matmul(
                    out=ps[:, 0:M],
                    lhsT=w_t[:, p],
                    rhs=f_q[:, pl, :, tb],
                    start=(p == 0),
                    stop=(p == NP - 1),
                    perf_mode=mybir.MatmulPerfMode.DoubleRow,
                )

        def stage_evac_tb(st, tb):
            s12 = spool.tile([2 * G, 16, V], bf16, tag=f"s12_tb{tb}", name=f"s12_tb{tb}")
            st[f"s12_{tb}"] = s12
            ps = st[f"ps{tb}"]
            psr = bass.AP(
                tensor=ps.tensor,
                offset=ps.offset,
                ap=[ps.ap[0], [V, 16], [1, V]],
            )
            nc.scalar.activation(
                out=s12,
                in_=psr,
                func=mybir.ActivationFunctionType.Copy,
                scale=1.0 / WSCALE,
                bias=c0_half,
            )

        def stage_scatter_tb(st, tb):
            s12t = tpool.tile(
                [128, 2, V], bf16, tag=f"s12t_tb{tb}", name=f"s12t_tb{tb}"
            )
            st[f"s12t_{tb}"] = s12t
            for o in range(2):
                # in [8(g), 16(tt), 18(v)] -> out [128(g,tt), 18]
                nc.scalar.dma_start(
                    out=s12t[:, o], in_=st[f"s12_{tb}"][o * G : (o + 1) * G]
                )

        def stage_chain_tb(st, tb):
            """score..out for one t-half of the last batch - all tiles are
            per-tb so the two half-pipelines share no tile-level deps."""
            s12t = st[f"s12t_{tb}"]
            sc = work.tile([128, V, V], bf16, tag=f"sc_tb{tb}", name=f"sc_tb{tb}")
            ex = work.tile([128, V * V], bf16, tag=f"ex_tb{tb}", name=f"ex_tb{tb}")
            exa = work.tile([128, V * V], bf16, tag=f"exa_tb{tb}", name=f"exa_tb{tb}")
            ssum = work.tile([128, V], f32, tag=f"ssum_tb{tb}", name=f"ssum_tb{tb}")
            rec = work.tile([128, V], bf16, tag=f"rec_tb{tb}", name=f"rec_tb{tb}")
            att = opool.tile([128, V, V], bf16, tag=f"att_tb{tb}", name=f"att_tb{tb}")
            s1b = bass.AP(
                tensor=s12t.tensor,
                offset=s12t.offset,
                ap=[s12t.ap[0], [1, V], [0, V]],
            )
            s2b = bass.AP(
                tensor=s12t.tensor,
                offset=s12t.offset + V,
                ap=[s12t.ap[0], [0, V], [1, V]],
            )
            nc.vector.tensor_add(out=sc, in0=s1b, in1=s2b)
            scf = bass.AP(
                tensor=sc.tensor, offset=sc.offset, ap=[sc.ap[0], [1, V * V]]
            )
            nc.vector.scalar_tensor_tensor(
                out=ex, in0=scf, scalar=0.1, in1=scf,
                op0=AluOpType.mult, op1=AluOpType.max,
            )
            nc.scalar.activation(
                out=ex, in_=ex, func=mybir.ActivationFunctionType.Exp
            )
            abc = bass.AP(
                tensor=a_bc.tensor, offset=a_bc.offset,
                ap=[a_bc.ap[0], [1, V * V]],
            )
            nc.vector.tensor_mul(out=exa, in0=ex, in1=abc)
            exr = bass.AP(
                tensor=exa.tensor, offset=exa.offset,
                ap=[exa.ap[0], [V, V], [1, V]],
            )
            nc.vector.reduce_sum(out=ssum, in_=exr, axis=mybir.AxisListType.X)
            with nc.allow_low_precision(reason="bf16 reciprocal is plenty here"):
                nc.vector.reciprocal(out=rec, in_=ssum)
            rb = bass.AP(
                tensor=rec.tensor, offset=rec.offset,
                ap=[rec.ap[0], [1, V], [0, V]],
            )
            nc.vector.tensor_mul(out=att, in0=exr, in1=rb)
            nc.sync.dma_start(
                out=out[st["b"], tb * 128 : (tb + 1) * 128, :, :],
                in_=att,
            )

        # Software pipeline.  All feat DMAs are issued up front (fpool holds
        # every half-batch tile); compute stages are emitted with explicit
        # lags so every engine's program-order stream matches data-arrival
        # order and no stream head-blocks on a later dependency.
        stages = [{"b": b} for b in range(BPC)]
        for b in range(BPC):
            stages[b]["f_t"] = stage_feat(b)

        def emit_front(k):
            """evac..exp for batch k (ready right after its matmuls)."""
            st = stages[k]
            stage_evac(st)
            stage_scatter(st)
            stage_score(st)
            stage_exp(st)

        def emit_tail(k):
            """mask..out for batch k."""
            st = stages[k]
            stage_mask(st, on_pool=(k < BPC - 1))
            stage_sum(st)
            stage_norm(st, on_pool=(k < BPC - 1))
            stage_out(st)

        LB = BPC - 1  # last batch runs at tb granularity
        for b in range(LB):
            stage_matmul(stages[b])
            if b >= 1:
                emit_front(b - 1)
            if b >= 2:
                emit_tail(b - 2)
        st = stages[LB]
        stage_matmul_tb(st, 0)
        emit_front(LB - 1)
        stage_evac_tb(st, 0)
        stage_scatter_tb(st, 0)
        stage_matmul_tb(st, 1)
        emit_tail(LB - 2)
        stage_evac_tb(st, 1)
        stage_scatter_tb(st, 1)
        emit_tail(LB - 1)
        stage_chain_tb(st, 0)
        stage_chain_tb(st, 1)
    return nc


def _prep_params(Wf, bf, Wa, ba):
    import ml_dtypes

    w1, w2 = Wa[:64].astype(np.float64), Wa[64:].astype(np.float64)
    Wf64, bf64 = Wf.astype(np.float64), bf.astype(np.float64)
    u = np.stack([w1 @ Wf64, w2 @ Wf64])  # [2, 256]
    c0 = float(w1 @ bf64 + w2 @ bf64 + float(ba[0]))
    # block-diagonal rotated weights: wmat[(g,c16), p, j, (o,g')] =
    #   u[o, ((g+p)%8)*32 + j*16 + c16] * WSCALE  if g'==g else 0
    # (columns o-major so psum rows for each o are contiguous)
    wm = np.zeros((G, 16, NP, 2, 2, G), dtype=np.float64)
    for g in range(G):
        for p in range(NP):
            sub = (g + p) % NP
            for j in range(2):
                cs = sub * 32 + j * 16 + np.arange(16)
                wm[g, :, p, j, :, g] = u[:, cs].T * WSCALE
    wmat = wm.reshape(128, NP, 2, 2 * G).astype(ml_dtypes.float8_e4m3)
    return wmat, c0 / 2.0


def _pack_feat(feat_core):
    """[bpc, 256, 256, 18] f32 -> [bpc, 128, NP, 2, 2, M] fp8e4 with the
    rotation pre-baked: part=(g,c16), free=(pass, pair, thalf, t16, v) holds
    feat[c = ((g+pass)%8)*32 + pair*16 + c16, t = thalf*128 + g*16 + t16, v].
    """
    import ml_dtypes

    bpc = feat_core.shape[0]
    a8 = feat_core.astype(ml_dtypes.float8_e4m3)
    # c = sub*32 + j*16 + c16 ; t = tb*128 + g*16 + t16
    a = a8.reshape(bpc, NP, 2, 16, 2, G, 16, V)  # b, sub, j, c16, tb, g, t16, v
    a = a.transpose(0, 5, 3, 1, 2, 4, 6, 7)  # b, g, c16, sub, j, tb, t16, v
    packed = np.empty((bpc, G, 16, NP, 2, 2, 16, V), dtype=ml_dtypes.float8_e4m3)
    for g in range(G):
        packed[:, g] = a[:, g][:, :, (g + np.arange(NP)) % NP]
    return np.ascontiguousarray(packed.reshape(bpc, 128, NP, 2, 2, M))


def get_nc(c0_half):
    global _cached_nc
    if _cached_nc is None:
        _cached_nc = _build_nc(c0_half)
    return _cached_nc


def kernel(feat, A, Wf, bf, Wa, ba):
    _install_wait_legalizer()
    from concourse.bass_utils import run_bass_kernel_spmd

    import ml_dtypes

    feat = np.asarray(feat, dtype=np.float32)
    A = (
        np.ascontiguousarray(np.asarray(A, dtype=np.float32))
        .reshape(1, V * V)
        .astype(ml_dtypes.bfloat16)
    )
    wmat, c0_half = _prep_params(
        np.asarray(Wf, np.float32),
        np.asarray(bf, np.float32),
        np.asarray(Wa, np.float32),
        np.asarray(ba, np.float32),
    )

    nc = get_nc(c0_half)
    in_maps = [
        {
            "feat": _pack_feat(feat[i * BPC : (i + 1) * BPC]),
            "wmat": wmat,
            "amat": A,
        }
        for i in range(NCORES)
    ]
    res = run_bass_kernel_spmd(nc, in_maps, core_ids=list(range(NCORES)))
    return np.concatenate(
        [np.asarray(r["out"]).astype(np.float32) for r in res.results], axis=0
    )
